# revision 10
# baseline (speedup 1.0000x reference)
"""BitNet attention Trainium2 kernel — 8-core SPMD (batch x head-group sharding).

Sharding: core c -> batch b = c//4, head-group g = c%4 (heads 4g..4g+3,
inner cols 256g..256g+256). Wq/Wk/Wv split column-wise over heads, Wo
row-wise; per-tensor absmean/absmax quantization scales are computed
on-device with AllReduce(add/max) collectives across the 8 cores.

All matmuls run in bf16 on the TensorEngine. This is exact for the
BitNet-quantized operands: int8 activation codes (|q| <= 127) and ternary
weights {-1,0,1} are exactly representable in bf16, and PSUM accumulates
in fp32 (integer sums < 2^24), so projection outputs equal the reference's
quantized matmuls up to final-scale rounding. Attention (RoPE'd q/k,
softmax weights, v) runs in bf16 with fp32 exp/accumulation: validated
end-to-end rel err ~3e-3 (attn_weights) / ~8e-3 (attn_output) vs the f32
reference.

Softmax row sums come free from the attn@v matmul: V carries shared
ones-blocks so each head's stationary operand is a contiguous
[128, 128] = [v_h | 1] (or [1 | v_h]) slab — the matmul emits attn_out
and the softmax row sums (broadcast over 64 partitions) in one pass at
full M=128 density. attn_weights are produced in [key, query] layout per
head and transposed on the host during unshard.
"""

import sys
import numpy as np

sys.path.insert(0, "/opt/trn_rl_repo")

B = 2
S_FULL = 2048
D = 1024
H = 16
DKV = 64
N_CORES = 8
HPC = 4              # heads per core
DC = HPC * DKV       # 256 inner dims per core
ROPE_BASE = 10000.0
MAGIC = 12582912.0   # 1.5 * 2^23: (x + MAGIC) - MAGIC == round-to-nearest-even(x)

# v_sm column layout (bf16, 8 x 64 = 512 cols):
#   [v0 | ones | v1 | pad | v2 | ones | v3 | pad]
V_DST = (0, 128, 256, 384)   # where head h's v columns go (stride 128)
V_LHS = (0, 64, 256, 320)    # head h's [v|1] / [1|v] stationary slab start

_CACHE = {}


def build_program(S=S_FULL, debug=False):
    import concourse.bass as bass
    import concourse.mybir as mybir
    import concourse.tile as tile
    import concourse.bass_isa as bass_isa
    from concourse import bacc

    fp32 = mybir.dt.float32
    bf16 = mybir.dt.bfloat16
    AX = mybir.AxisListType
    OP = mybir.AluOpType
    ACTF = mybir.ActivationFunctionType

    KC = S // 128            # key chunks
    QW = min(512, S)         # matmul moving free dim
    NQ = S // QW
    EXPW = min(1024, S)      # width per scores-psum tile / exp call
    NE = S // EXPW

    nc = bacc.Bacc(num_devices=N_CORES)

    xT = nc.declare_dram_parameter("xT", [D, S], fp32, isOutput=False)
    wq = nc.declare_dram_parameter("wq", [128, 8 * DC], fp32, isOutput=False)
    wk = nc.declare_dram_parameter("wk", [128, 8 * DC], fp32, isOutput=False)
    wv = nc.declare_dram_parameter("wv", [128, 8 * DC], fp32, isOutput=False)
    wo = nc.declare_dram_parameter("wo", [DC, D], fp32, isOutput=False)
    cosT = nc.declare_dram_parameter("cosT", [128, S], fp32, isOutput=False)
    sinT = nc.declare_dram_parameter("sinT", [128, S], fp32, isOutput=False)

    attw = nc.declare_dram_parameter("attw", [HPC, S, S], bf16, isOutput=True)
    yT = nc.declare_dram_parameter("yT", [D, S], fp32, isOutput=True)
    if debug:
        d_scB = nc.declare_dram_parameter("d_scB", [128, 8], fp32, isOutput=True)
        d_xq = nc.declare_dram_parameter("d_xq", [128, S], fp32, isOutput=True)
        d_qi = nc.declare_dram_parameter("d_qi", [128, S], fp32, isOutput=True)
        d_qr = nc.declare_dram_parameter("d_qr", [128, S], fp32, isOutput=True)
        d_v = nc.declare_dram_parameter("d_v", [128, 512], fp32, isOutput=True)
        d_exp = nc.declare_dram_parameter("d_exp", [128, S], fp32, isOutput=True)
        d_rc = nc.declare_dram_parameter("d_rc", [64, S], fp32, isOutput=True)
        d_ao = nc.declare_dram_parameter("d_ao", [128, S], fp32, isOutput=True)
        d_wb = nc.declare_dram_parameter("d_wb", [128, 8 * DC], fp32, isOutput=True)

    b_ws_in = nc.dram_tensor("b_ws_in", [1, 4], fp32)
    b_ws_out = nc.dram_tensor("b_ws_out", [1, 4], fp32, addr_space="Shared")
    b_xm_in = nc.dram_tensor("b_xm_in", [1, 1], fp32)
    b_xm_out = nc.dram_tensor("b_xm_out", [1, 1], fp32, addr_space="Shared")
    b_am_in = nc.dram_tensor("b_am_in", [1, 1], fp32)
    b_am_out = nc.dram_tensor("b_am_out", [1, 1], fp32, addr_space="Shared")
    RG = [list(range(N_CORES))]

    W_EXT = {"q": wq, "k": wk, "v": wv}

    with tile.TileContext(nc) as tc:
        with (
            tc.tile_pool(name="persist", bufs=1) as pp,
            tc.tile_pool(name="smalls", bufs=1) as mp,
        ):
            # kernel-lifetime tiles
            wo_bf = [pp.tile([128, D], bf16, tag=f"wbo{k}", name=f"wbo{k}")
                     for k in range(2)]
            rope_bf = {n: (pp.tile([128, S], bf16, tag=f"cos{n}", name=f"cos{n}"),
                           pp.tile([128, S], bf16, tag=f"sin{n}", name=f"sin{n}"))
                       for n in ("q", "k")}
            qr = [pp.tile([128, S], bf16, tag=f"qr{t}", name=f"qr{t}")
                  for t in range(2)]
            kr = [pp.tile([128, S], bf16, tag=f"kr{t}", name=f"kr{t}")
                  for t in range(2)]
            v_sm = [pp.tile([128, 512], bf16, tag=f"v{c}", name=f"v{c}")
                    for c in range(KC)]
            att_oT = [pp.tile([128, S], fp32, tag=f"ao{t}", name=f"ao{t}")
                      for t in range(2)]
            aoq = [pp.tile([128, S], bf16, tag=f"aoq{t}", name=f"aoq{t}")
                   for t in range(2)]

            # ---------- pass 1: streaming reduces for the scales ----------
            with tc.tile_pool(name="stream", bufs=3) as st:
                wsum_cols = mp.tile([128, 4], fp32)
                wo_parts = mp.tile([128, 2], fp32)
                for i, n in enumerate(("q", "k", "v")):
                    wt = st.tile([128, 8 * DC], fp32, tag="st32", name="wt")
                    nc.sync.dma_start(wt[:], W_EXT[n][:])
                    nc.vector.tensor_reduce(
                        wsum_cols[:, i : i + 1], wt[:], axis=AX.X, op=OP.add,
                        apply_absolute_value=True)
                for k in range(2):
                    wt = st.tile([128, D], fp32, tag="st32", name="wt")
                    nc.sync.dma_start(wt[:], wo[k * 128 : (k + 1) * 128, :])
                    nc.vector.tensor_reduce(
                        wo_parts[:, k : k + 1], wt[:], axis=AX.X, op=OP.add,
                        apply_absolute_value=True)
                nc.vector.tensor_reduce(
                    wsum_cols[:, 3:4], wo_parts[:], axis=AX.X, op=OP.add)
                wsum_r = mp.tile([128, 4], fp32)
                nc.gpsimd.partition_all_reduce(
                    wsum_r[:], wsum_cols[:], channels=128,
                    reduce_op=bass_isa.ReduceOp.add)
                ws_in_sb = mp.tile([1, 4], fp32)
                nc.vector.tensor_copy(ws_in_sb[:], wsum_r[0:1, :])
                nc.sync.dma_start(b_ws_in[:], ws_in_sb[:])
                nc.gpsimd.collective_compute(
                    "AllReduce", OP.add, replica_groups=RG,
                    ins=[b_ws_in[:]], outs=[b_ws_out[:]])

                xmax_cols = mp.tile([128, 8], fp32)
                for i in range(8):
                    xt = st.tile([128, S], fp32, tag="st32", name="xt")
                    nc.sync.dma_start(xt[:], xT[i * 128 : (i + 1) * 128, :])
                    nc.vector.tensor_reduce(
                        xmax_cols[:, i : i + 1], xt[:], axis=AX.X, op=OP.max,
                        apply_absolute_value=True)
                xmax_1 = mp.tile([128, 1], fp32)
                nc.vector.tensor_reduce(
                    xmax_1[:], xmax_cols[:], axis=AX.X, op=OP.max)
                xmax_r = mp.tile([128, 1], fp32)
                nc.gpsimd.partition_all_reduce(
                    xmax_r[:], xmax_1[:], channels=128,
                    reduce_op=bass_isa.ReduceOp.max)
                xm_in_sb = mp.tile([1, 1], fp32)
                nc.vector.tensor_copy(xm_in_sb[:], xmax_r[0:1, :])
                nc.sync.dma_start(b_xm_in[:], xm_in_sb[:])
                nc.gpsimd.collective_compute(
                    "AllReduce", OP.max, replica_groups=RG,
                    ins=[b_xm_in[:]], outs=[b_xm_out[:]])

                # ---------- scalars ----------
                ws_sb = mp.tile([1, 4], fp32)
                nc.sync.dma_start(ws_sb[:], b_ws_out[:])
                xm_sb = mp.tile([1, 1], fp32)
                nc.sync.dma_start(xm_sb[:], b_xm_out[:])
                sw_sb = mp.tile([1, 4], fp32)
                nc.vector.tensor_scalar_mul(
                    sw_sb[:], ws_sb[:], 1.0 / (2.0 * D * D))
                rw_sb = mp.tile([1, 4], fp32)
                nc.vector.reciprocal(rw_sb[:], sw_sb[:])
                sx_sb = mp.tile([1, 1], fp32)
                nc.vector.tensor_scalar_mul(sx_sb[:], xm_sb[:], 1.0 / 127.0)
                rx_sb = mp.tile([1, 1], fp32)
                nc.vector.reciprocal(rx_sb[:], sx_sb[:])
                tab_sb = mp.tile([1, 3], fp32)
                nc.vector.tensor_mul(
                    tab_sb[:], sw_sb[:, 0:3],
                    sx_sb[0:1, 0:1].broadcast_to([1, 3]))
                # cols: 0=r_x 1=rw_q 2=rw_k 3=rw_v 4=rw_o 5=tab_q 6=tab_k 7=sc_v
                scv = mp.tile([1, 8], fp32)
                nc.vector.tensor_copy(scv[:, 0:1], rx_sb[:])
                nc.vector.tensor_copy(scv[:, 1:5], rw_sb[:])
                nc.vector.tensor_copy(scv[:, 5:8], tab_sb[:])
                scB = mp.tile([128, 8], fp32)
                nc.gpsimd.partition_broadcast(scB[:], scv[:], channels=128)
                if debug:
                    nc.sync.dma_start(d_scB[:], scB[:])

            # ---------- pass 2: re-load + quantize ----------
            with (
                tc.tile_pool(name="mid", bufs=1) as pm,
                tc.tile_pool(name="stream2", bufs=3) as st,
            ):
                w_bf = {}
                for i, n in enumerate(("q", "k", "v")):
                    wt = st.tile([128, 8 * DC], fp32, tag="st32", name="wt")
                    nc.sync.dma_start(wt[:], W_EXT[n][:])
                    t1 = st.tile([128, 8 * DC], fp32, tag="st32", name="t1")
                    nc.vector.tensor_scalar(
                        t1[:], wt[:], scB[:, i + 1 : i + 2], MAGIC,
                        op0=OP.mult, op1=OP.add)
                    t2 = st.tile([128, 8 * DC], fp32, tag="st32", name="t2")
                    nc.vector.tensor_scalar_add(t2[:], t1[:], -MAGIC)
                    wb = pm.tile([128, 8 * DC], bf16, tag=f"wb{n}", name=f"wb{n}")
                    nc.vector.tensor_scalar(
                        wb[:], t2[:], 1.0, -1.0, op0=OP.min, op1=OP.max)
                    w_bf[n] = wb
                    if debug and n == "q":
                        nc.gpsimd.dma_start(d_wb[:], wb[:])
                for k in range(2):
                    wt = st.tile([128, D], fp32, tag="st32", name="wt")
                    nc.sync.dma_start(wt[:], wo[k * 128 : (k + 1) * 128, :])
                    t1 = st.tile([128, D], fp32, tag="st32", name="t1")
                    nc.vector.tensor_scalar(
                        t1[:], wt[:], scB[:, 4:5], MAGIC, op0=OP.mult, op1=OP.add)
                    t2 = st.tile([128, D], fp32, tag="st32", name="t2")
                    nc.vector.tensor_scalar_add(t2[:], t1[:], -MAGIC)
                    nc.vector.tensor_scalar(
                        wo_bf[k][:], t2[:], 1.0, -1.0, op0=OP.min, op1=OP.max)

                xq = []
                for i in range(8):
                    xt = st.tile([128, S], fp32, tag="st32", name="xt")
                    nc.sync.dma_start(xt[:], xT[i * 128 : (i + 1) * 128, :])
                    t1 = st.tile([128, S], fp32, tag="st32", name="t1")
                    nc.vector.tensor_scalar(
                        t1[:], xt[:], scB[:, 0:1], MAGIC, op0=OP.mult, op1=OP.add)
                    xb = pm.tile([128, S], bf16, tag=f"xq{i}", name=f"xq{i}")
                    nc.vector.tensor_scalar_add(xb[:], t1[:], -MAGIC)
                    xq.append(xb)
                    if debug and i == 0:
                        nc.gpsimd.dma_start(d_xq[:], xb[:])

                # scaled rope tables (bf16)
                for j, n in enumerate(("q", "k")):
                    for src_ext, dst in ((cosT, rope_bf[n][0]),
                                         (sinT, rope_bf[n][1])):
                        tf = st.tile([128, S], fp32, tag="st32", name="tf")
                        nc.sync.dma_start(tf[:], src_ext[:])
                        nc.vector.tensor_scalar_mul(
                            dst[:], tf[:], scB[:, 5 + j : 6 + j])

                # ---------- qkv projections ----------
                q_int = [pm.tile([128, S], bf16, tag=f"qi{t}", name=f"qi{t}")
                         for t in range(2)]
                k_int = [pm.tile([128, S], bf16, tag=f"ki{t}", name=f"ki{t}")
                         for t in range(2)]
                with tc.tile_pool(name="psq", bufs=4, space="PSUM") as psq:
                    for n, dest in (("q", q_int), ("k", k_int)):
                        for m in range(2):
                            for nn in range(NQ):
                                ps = psq.tile([128, QW], fp32, tag="ps", name="ps")
                                for kk in range(8):
                                    nc.tensor.matmul(
                                        ps[:],
                                        w_bf[n][:, kk * DC + m * 128
                                                : kk * DC + (m + 1) * 128],
                                        xq[kk][:, nn * QW : (nn + 1) * QW],
                                        start=(kk == 0), stop=(kk == 7),
                                    )
                                nc.vector.tensor_copy(
                                    dest[m][:, nn * QW : (nn + 1) * QW], ps[:])
                    for ci in range(KC):
                        ps = psq.tile([128, DC], fp32, tag="psv", name="psv")
                        for kk in range(8):
                            nc.tensor.matmul(
                                ps[:],
                                xq[kk][:, ci * 128 : (ci + 1) * 128],
                                w_bf["v"][:, kk * DC : (kk + 1) * DC],
                                start=(kk == 0), stop=(kk == 7),
                            )
                        vt = v_sm[ci]
                        for hh in range(HPC):
                            nc.vector.tensor_scalar_mul(
                                vt[:, V_DST[hh] : V_DST[hh] + 64],
                                ps[:, hh * 64 : (hh + 1) * 64], scB[:, 7:8])
                        if debug and ci == 0:
                            nc.gpsimd.dma_start(d_v[:], vt[:])
                    for ci in range(KC):
                        nc.vector.memset(v_sm[ci][:, 64:128], 1.0)
                        nc.vector.memset(v_sm[ci][:, 320:384], 1.0)

                # ---------- rope (bf16) ----------
                for n, src, dst in (("q", q_int, qr), ("k", k_int, kr)):
                    cb, sb = rope_bf[n]
                    for t in range(2):
                        sh = st.tile([128, S], bf16, tag="stbf", name="sh")
                        for blk in range(4):
                            sblk = (blk ^ 1) * 32
                            nc.sync.dma_start(
                                sh[blk * 32 : blk * 32 + 32, :],
                                src[t][sblk : sblk + 32, :])
                        t1 = st.tile([128, S], bf16, tag="stbf", name="t1b")
                        nc.vector.tensor_mul(t1[:], src[t][:], cb[:])
                        t2 = st.tile([128, S], bf16, tag="stbf", name="t2b")
                        nc.vector.tensor_mul(t2[:], sh[:], sb[:])
                        nc.vector.tensor_add(dst[t][:], t1[:], t2[:])
                        if debug and n == "q" and t == 0:
                            nc.gpsimd.dma_start(d_qi[:], src[t][:])
                            nc.gpsimd.dma_start(d_qr[:], dst[t][:])

            # ---------- attention ----------
            with (
                tc.tile_pool(name="pss", bufs=2, space="PSUM") as pss,
                tc.tile_pool(name="psav", bufs=1, space="PSUM") as psav,
                tc.tile_pool(name="expp", bufs=KC + 2) as expp,
                tc.tile_pool(name="wnp", bufs=3) as wnp,
                tc.tile_pool(name="rsp", bufs=1) as rsp,
            ):
                for h in range(HPC):
                    tp, half = h // 2, (h % 2) * 64
                    ao_base, sum_base = (0, 64) if h % 2 == 0 else (64, 0)
                    pav = psav.tile([128, S], fp32, tag="pav", name="pav")
                    exp_tiles = []
                    for kc in range(KC):
                        et = expp.tile([128, S], bf16, tag="exp", name=f"exp{kc}")
                        exp_tiles.append(et)
                        for ne in range(NE):
                            ps = pss.tile([128, EXPW], fp32, tag="pss", name="ps")
                            for nqq in range(EXPW // QW):
                                n = ne * (EXPW // QW) + nqq
                                nc.tensor.matmul(
                                    ps[:, nqq * QW : (nqq + 1) * QW],
                                    kr[tp][half : half + 64,
                                           kc * 128 : (kc + 1) * 128],
                                    qr[tp][half : half + 64,
                                           n * QW : (n + 1) * QW],
                                    start=True, stop=True,
                                )
                            nc.scalar.activation(
                                et[:, ne * EXPW : (ne + 1) * EXPW], ps[:],
                                ACTF.Exp, scale=0.125)
                            if debug and h == 0 and kc == 0 and ne == 0:
                                nc.gpsimd.dma_start(d_exp[:, 0 : EXPW], et[:, 0 : EXPW])
                        for n in range(NQ):
                            nc.tensor.matmul(
                                pav[:, n * QW : (n + 1) * QW],
                                v_sm[kc][:, V_LHS[h] : V_LHS[h] + 128],
                                et[:, n * QW : (n + 1) * QW],
                                start=(kc == 0), stop=(kc == KC - 1),
                            )
                    rs64 = rsp.tile([64, S], fp32, tag="rs64", name="rs64")
                    nc.scalar.copy(rs64[:], pav[sum_base : sum_base + 64, :])
                    rc64 = rsp.tile([64, S], fp32, tag="rc64", name="rc64")
                    rscr = rsp.tile([64, S], fp32, tag="rscr", name="rscr")
                    nc.vector.reciprocal_approx_accurate(
                        rc64[:], rs64[:], rscr[:])
                    if debug and h == 0:
                        nc.sync.dma_start(d_rc[:], rc64[:])
                    nc.vector.tensor_mul(
                        att_oT[tp][half : half + 64, :],
                        pav[ao_base : ao_base + 64, :], rc64[:])
                    rcb = rsp.tile([128, S], bf16, tag="rcb", name="rcb")
                    if debug and h == 1:
                        nc.sync.dma_start(d_ao[:], att_oT[0][:])
                    nc.vector.tensor_copy(rcb[0:64, :], rc64[:])
                    nc.vector.tensor_copy(rcb[64:128, :], rc64[:])
                    for kc in range(KC):
                        wn = wnp.tile([128, S], bf16, tag="wn", name="wn")
                        nc.vector.tensor_mul(wn[:], exp_tiles[kc][:], rcb[:])
                        nc.sync.dma_start(
                            attw[h, kc * 128 : (kc + 1) * 128, :], wn[:])

            # ---------- attn_out quantization ----------
            am_cols = mp.tile([128, 2], fp32)
            for t in range(2):
                nc.vector.tensor_reduce(
                    am_cols[:, t : t + 1], att_oT[t][:], axis=AX.X, op=OP.max,
                    apply_absolute_value=True)
            am1 = mp.tile([128, 1], fp32)
            nc.vector.tensor_reduce(am1[:], am_cols[:], axis=AX.X, op=OP.max)
            am_r = mp.tile([128, 1], fp32)
            nc.gpsimd.partition_all_reduce(
                am_r[:], am1[:], channels=128, reduce_op=bass_isa.ReduceOp.max)
            am_in_sb = mp.tile([1, 1], fp32)
            nc.vector.tensor_copy(am_in_sb[:], am_r[0:1, :])
            nc.sync.dma_start(b_am_in[:], am_in_sb[:])
            nc.gpsimd.collective_compute(
                "AllReduce", OP.max, replica_groups=RG,
                ins=[b_am_in[:]], outs=[b_am_out[:]])
            am_sb = mp.tile([1, 1], fp32)
            nc.sync.dma_start(am_sb[:], b_am_out[:])

            sao_sb = mp.tile([1, 1], fp32)
            nc.vector.tensor_scalar_mul(sao_sb[:], am_sb[:], 1.0 / 127.0)
            rao_sb = mp.tile([1, 1], fp32)
            nc.vector.reciprocal(rao_sb[:], sao_sb[:])
            scv2 = mp.tile([1, 2], fp32)
            nc.vector.tensor_copy(scv2[:, 0:1], rao_sb[:])
            nc.vector.tensor_mul(scv2[:, 1:2], sao_sb[:], sw_sb[:, 3:4])
            scB2 = mp.tile([128, 2], fp32)
            nc.gpsimd.partition_broadcast(scB2[:], scv2[:], channels=128)

            with tc.tile_pool(name="tail", bufs=2) as tl:
                for t in range(2):
                    t1 = tl.tile([128, S], fp32, tag="aotmp", name="aot")
                    nc.vector.tensor_scalar(
                        t1[:], att_oT[t][:], scB2[:, 0:1], MAGIC,
                        op0=OP.mult, op1=OP.add)
                    nc.vector.tensor_scalar_add(aoq[t][:], t1[:], -MAGIC)

                # ---------- output projection (partial sums) ----------
                with tc.tile_pool(name="psw", bufs=4, space="PSUM") as psw:
                    for m in range(8):
                        ys = tl.tile([128, S], fp32, tag="ys", name="ys")
                        for n in range(NQ):
                            ps = psw.tile([128, QW], fp32, tag="psw", name="ps")
                            for kk in range(2):
                                nc.tensor.matmul(
                                    ps[:],
                                    wo_bf[kk][:, m * 128 : (m + 1) * 128],
                                    aoq[kk][:, n * QW : (n + 1) * QW],
                                    start=(kk == 0), stop=(kk == 1),
                                )
                            nc.vector.tensor_scalar_mul(
                                ys[:, n * QW : (n + 1) * QW], ps[:], scB2[:, 1:2])
                        nc.sync.dma_start(yT[m * 128 : (m + 1) * 128, :], ys[:])

    nc.compile()
    return nc


def _prepare_inputs(hidden_states, Wq, Wk, Wv, Wo, S):
    inv_freq = 1.0 / (ROPE_BASE ** (np.arange(0, DKV, 2, dtype=np.float32) / DKV))
    t = np.arange(S, dtype=np.float32)
    freqs = np.einsum("i,j->ij", t, inv_freq).astype(np.float32)
    emb = np.concatenate([freqs, freqs], axis=-1)
    cosT1 = np.cos(emb).T.astype(np.float32)
    sinT1 = np.sin(emb).T.astype(np.float32)
    sgn = np.where(np.arange(DKV) < DKV // 2, -1.0, 1.0).astype(np.float32)
    sinT1 = sinT1 * sgn[:, None]
    cos2 = np.ascontiguousarray(np.concatenate([cosT1, cosT1], axis=0))
    sin2 = np.ascontiguousarray(np.concatenate([sinT1, sinT1], axis=0))

    def pack_w(wT):  # [1024, 256] -> [128, 8*256] chunk-interleaved
        return np.ascontiguousarray(
            wT.reshape(8, 128, DC).transpose(1, 0, 2).reshape(128, 8 * DC))

    in_maps = []
    for c in range(N_CORES):
        b, g = c // 4, c % 4
        cols = slice(g * DC, (g + 1) * DC)
        in_maps.append({
            "xT": np.ascontiguousarray(hidden_states[b].T),
            "wq": pack_w(np.ascontiguousarray(Wq[cols, :].T)),
            "wk": pack_w(np.ascontiguousarray(Wk[cols, :].T)),
            "wv": pack_w(np.ascontiguousarray(Wv[cols, :].T)),
            "wo": np.ascontiguousarray(Wo[:, cols].T),
            "cosT": cos2,
            "sinT": sin2,
        })
    return in_maps


def run_sharded(hidden_states, Wq, Wk, Wv, Wo, trace=False):
    from concourse.bass_utils import run_bass_kernel_spmd

    Sv = hidden_states.shape[1]
    if "nc" not in _CACHE or _CACHE.get("S") != Sv:
        _CACHE["nc"] = build_program(Sv)
        _CACHE["S"] = Sv
    nc = _CACHE["nc"]
    in_maps = _prepare_inputs(hidden_states, Wq, Wk, Wv, Wo, Sv)
    res = run_bass_kernel_spmd(nc, in_maps, list(range(N_CORES)), trace=trace)

    attn_w = np.empty((B, H, Sv, Sv), dtype=np.float32)
    out = np.zeros((B, Sv, D), dtype=np.float32)
    for c in range(N_CORES):
        b, g = c // 4, c % 4
        aw = np.asarray(res.results[c]["attw"]).astype(np.float32)  # [4, Sk, Sq]
        for h in range(HPC):
            attn_w[b, g * HPC + h] = aw[h].T
        out[b] += np.asarray(res.results[c]["yT"]).T
    return (out, attn_w), res


def kernel(hidden_states, Wq, Wk, Wv, Wo):
    hidden_states = np.asarray(hidden_states, dtype=np.float32)
    (out, attn_w), _ = run_sharded(
        hidden_states,
        np.asarray(Wq, dtype=np.float32), np.asarray(Wk, dtype=np.float32),
        np.asarray(Wv, dtype=np.float32), np.asarray(Wo, dtype=np.float32),
    )
    return out, attn_w


# revision 11
# speedup vs baseline: 1.0099x; 1.0099x over previous
"""BitNet attention Trainium2 kernel — 8-core SPMD (batch x head-group sharding).

Sharding: core c -> batch b = c//4, head-group g = c%4 (heads 4g..4g+3,
inner cols 256g..256g+256). Wq/Wk/Wv split column-wise over heads, Wo
row-wise; per-tensor absmean/absmax quantization scales are computed
on-device with AllReduce(add/max) collectives across the 8 cores.

All matmuls run in bf16 on the TensorEngine. This is exact for the
BitNet-quantized operands: int8 activation codes (|q| <= 127) and ternary
weights {-1,0,1} are exactly representable in bf16, and PSUM accumulates
in fp32 (integer sums < 2^24), so projection outputs equal the reference's
quantized matmuls up to final-scale rounding. Attention (RoPE'd q/k,
softmax weights, v) runs in bf16 with fp32 exp/accumulation: validated
end-to-end rel err ~3e-3 (attn_weights) / ~8e-3 (attn_output) vs the f32
reference.

Softmax row sums come free from the attn@v matmul: V carries shared
ones-blocks so each head's stationary operand is a contiguous
[128, 128] = [v_h | 1] (or [1 | v_h]) slab — the matmul emits attn_out
and the softmax row sums (broadcast over 64 partitions) in one pass at
full M=128 density. attn_weights are produced in [key, query] layout per
head and transposed on the host during unshard.
"""

import sys
import numpy as np

sys.path.insert(0, "/opt/trn_rl_repo")

B = 2
S_FULL = 2048
D = 1024
H = 16
DKV = 64
N_CORES = 8
HPC = 4              # heads per core
DC = HPC * DKV       # 256 inner dims per core
ROPE_BASE = 10000.0
MAGIC = 12582912.0   # 1.5 * 2^23: (x + MAGIC) - MAGIC == round-to-nearest-even(x)

# v_sm column layout (bf16, 8 x 64 = 512 cols):
#   [v0 | ones | v1 | pad | v2 | ones | v3 | pad]
V_DST = (0, 128, 256, 384)   # where head h's v columns go (stride 128)
V_LHS = (0, 64, 256, 320)    # head h's [v|1] / [1|v] stationary slab start

_CACHE = {}


def build_program(S=S_FULL, debug=False):
    import concourse.bass as bass
    import concourse.mybir as mybir
    import concourse.tile as tile
    import concourse.bass_isa as bass_isa
    from concourse import bacc

    fp32 = mybir.dt.float32
    bf16 = mybir.dt.bfloat16
    AX = mybir.AxisListType
    OP = mybir.AluOpType
    ACTF = mybir.ActivationFunctionType

    KC = S // 128            # key chunks
    QW = min(512, S)         # matmul moving free dim
    NQ = S // QW
    EXPW = min(1024, S)      # width per scores-psum tile / exp call
    NE = S // EXPW

    nc = bacc.Bacc(num_devices=N_CORES)

    xT = nc.declare_dram_parameter("xT", [D, S], fp32, isOutput=False)
    wq = nc.declare_dram_parameter("wq", [128, 8 * DC], fp32, isOutput=False)
    wk = nc.declare_dram_parameter("wk", [128, 8 * DC], fp32, isOutput=False)
    wv = nc.declare_dram_parameter("wv", [128, 8 * DC], fp32, isOutput=False)
    wo = nc.declare_dram_parameter("wo", [DC, D], fp32, isOutput=False)
    cosT = nc.declare_dram_parameter("cosT", [128, S], fp32, isOutput=False)
    sinT = nc.declare_dram_parameter("sinT", [128, S], fp32, isOutput=False)

    attw = nc.declare_dram_parameter("attw", [HPC, S, S], bf16, isOutput=True)
    yT = nc.declare_dram_parameter("yT", [D, S], fp32, isOutput=True)
    if debug:
        d_scB = nc.declare_dram_parameter("d_scB", [128, 8], fp32, isOutput=True)
        d_xq = nc.declare_dram_parameter("d_xq", [128, S], fp32, isOutput=True)
        d_qi = nc.declare_dram_parameter("d_qi", [128, S], fp32, isOutput=True)
        d_qr = nc.declare_dram_parameter("d_qr", [128, S], fp32, isOutput=True)
        d_v = nc.declare_dram_parameter("d_v", [128, 512], fp32, isOutput=True)
        d_exp = nc.declare_dram_parameter("d_exp", [128, S], fp32, isOutput=True)
        d_rc = nc.declare_dram_parameter("d_rc", [64, S], fp32, isOutput=True)
        d_ao = nc.declare_dram_parameter("d_ao", [128, S], fp32, isOutput=True)
        d_wb = nc.declare_dram_parameter("d_wb", [128, 8 * DC], fp32, isOutput=True)

    b_ws_in = nc.dram_tensor("b_ws_in", [1, 4], fp32)
    b_ws_out = nc.dram_tensor("b_ws_out", [1, 4], fp32, addr_space="Shared")
    b_xm_in = nc.dram_tensor("b_xm_in", [1, 1], fp32)
    b_xm_out = nc.dram_tensor("b_xm_out", [1, 1], fp32, addr_space="Shared")
    b_am_in = nc.dram_tensor("b_am_in", [1, 1], fp32)
    b_am_out = nc.dram_tensor("b_am_out", [1, 1], fp32, addr_space="Shared")
    RG = [list(range(N_CORES))]

    W_EXT = {"q": wq, "k": wk, "v": wv}

    with tile.TileContext(nc) as tc:
        with (
            tc.tile_pool(name="persist", bufs=1) as pp,
            tc.tile_pool(name="smalls", bufs=1) as mp,
        ):
            # kernel-lifetime tiles
            wo_bf = [pp.tile([128, D], bf16, tag=f"wbo{k}", name=f"wbo{k}")
                     for k in range(2)]
            rope_bf = {n: (pp.tile([128, S], bf16, tag=f"cos{n}", name=f"cos{n}"),
                           pp.tile([128, S], bf16, tag=f"sin{n}", name=f"sin{n}"))
                       for n in ("q", "k")}
            qr = [pp.tile([128, S], bf16, tag=f"qr{t}", name=f"qr{t}")
                  for t in range(2)]
            kr = [pp.tile([128, S], bf16, tag=f"kr{t}", name=f"kr{t}")
                  for t in range(2)]
            v_sm = [pp.tile([128, 512], bf16, tag=f"v{c}", name=f"v{c}")
                    for c in range(KC)]
            att_oT = [pp.tile([128, S], fp32, tag=f"ao{t}", name=f"ao{t}")
                      for t in range(2)]
            aoq = [pp.tile([128, S], bf16, tag=f"aoq{t}", name=f"aoq{t}")
                   for t in range(2)]

            # ---------- pass 1: streaming reduces for the scales ----------
            with tc.tile_pool(name="stream", bufs=3) as st:
                wsum_cols = mp.tile([128, 4], fp32)
                wo_parts = mp.tile([128, 2], fp32)
                for i, n in enumerate(("q", "k", "v")):
                    wt = st.tile([128, 8 * DC], fp32, tag="st32", name="wt")
                    nc.sync.dma_start(wt[:], W_EXT[n][:])
                    nc.vector.tensor_reduce(
                        wsum_cols[:, i : i + 1], wt[:], axis=AX.X, op=OP.add,
                        apply_absolute_value=True)
                for k in range(2):
                    wt = st.tile([128, D], fp32, tag="st32", name="wt")
                    nc.sync.dma_start(wt[:], wo[k * 128 : (k + 1) * 128, :])
                    nc.vector.tensor_reduce(
                        wo_parts[:, k : k + 1], wt[:], axis=AX.X, op=OP.add,
                        apply_absolute_value=True)
                nc.vector.tensor_reduce(
                    wsum_cols[:, 3:4], wo_parts[:], axis=AX.X, op=OP.add)
                wsum_r = mp.tile([128, 4], fp32)
                nc.gpsimd.partition_all_reduce(
                    wsum_r[:], wsum_cols[:], channels=128,
                    reduce_op=bass_isa.ReduceOp.add)
                ws_in_sb = mp.tile([1, 4], fp32)
                nc.vector.tensor_copy(ws_in_sb[:], wsum_r[0:1, :])
                nc.sync.dma_start(b_ws_in[:], ws_in_sb[:])
                nc.gpsimd.collective_compute(
                    "AllReduce", OP.add, replica_groups=RG,
                    ins=[b_ws_in[:]], outs=[b_ws_out[:]])

                xmax_cols = mp.tile([128, 8], fp32)
                for i in range(8):
                    xt = st.tile([128, S], fp32, tag="st32", name="xt")
                    nc.sync.dma_start(xt[:], xT[i * 128 : (i + 1) * 128, :])
                    nc.vector.tensor_reduce(
                        xmax_cols[:, i : i + 1], xt[:], axis=AX.X, op=OP.max,
                        apply_absolute_value=True)
                xmax_1 = mp.tile([128, 1], fp32)
                nc.vector.tensor_reduce(
                    xmax_1[:], xmax_cols[:], axis=AX.X, op=OP.max)
                xmax_r = mp.tile([128, 1], fp32)
                nc.gpsimd.partition_all_reduce(
                    xmax_r[:], xmax_1[:], channels=128,
                    reduce_op=bass_isa.ReduceOp.max)
                xm_in_sb = mp.tile([1, 1], fp32)
                nc.vector.tensor_copy(xm_in_sb[:], xmax_r[0:1, :])
                nc.sync.dma_start(b_xm_in[:], xm_in_sb[:])
                nc.gpsimd.collective_compute(
                    "AllReduce", OP.max, replica_groups=RG,
                    ins=[b_xm_in[:]], outs=[b_xm_out[:]])

                # ---------- scalars ----------
                ws_sb = mp.tile([1, 4], fp32)
                nc.sync.dma_start(ws_sb[:], b_ws_out[:])
                xm_sb = mp.tile([1, 1], fp32)
                nc.sync.dma_start(xm_sb[:], b_xm_out[:])
                sw_sb = mp.tile([1, 4], fp32)
                nc.vector.tensor_scalar_mul(
                    sw_sb[:], ws_sb[:], 1.0 / (2.0 * D * D))
                rw_sb = mp.tile([1, 4], fp32)
                nc.vector.reciprocal(rw_sb[:], sw_sb[:])
                sx_sb = mp.tile([1, 1], fp32)
                nc.vector.tensor_scalar_mul(sx_sb[:], xm_sb[:], 1.0 / 127.0)
                rx_sb = mp.tile([1, 1], fp32)
                nc.vector.reciprocal(rx_sb[:], sx_sb[:])
                tab_sb = mp.tile([1, 3], fp32)
                nc.vector.tensor_mul(
                    tab_sb[:], sw_sb[:, 0:3],
                    sx_sb[0:1, 0:1].broadcast_to([1, 3]))
                # cols: 0=r_x 1=rw_q 2=rw_k 3=rw_v 4=rw_o 5=tab_q 6=tab_k 7=sc_v
                scv = mp.tile([1, 8], fp32)
                nc.vector.tensor_copy(scv[:, 0:1], rx_sb[:])
                nc.vector.tensor_copy(scv[:, 1:5], rw_sb[:])
                nc.vector.tensor_copy(scv[:, 5:8], tab_sb[:])
                scB = mp.tile([128, 8], fp32)
                nc.gpsimd.partition_broadcast(scB[:], scv[:], channels=128)
                if debug:
                    nc.sync.dma_start(d_scB[:], scB[:])

            # ---------- pass 2: re-load + quantize ----------
            with (
                tc.tile_pool(name="mid", bufs=1) as pm,
                tc.tile_pool(name="stream2", bufs=3) as st,
            ):
                w_bf = {}
                for i, n in enumerate(("q", "k", "v")):
                    wt = st.tile([128, 8 * DC], fp32, tag="st32", name="wt")
                    nc.sync.dma_start(wt[:], W_EXT[n][:])
                    t1 = st.tile([128, 8 * DC], fp32, tag="st32", name="t1")
                    nc.vector.tensor_scalar(
                        t1[:], wt[:], scB[:, i + 1 : i + 2], MAGIC,
                        op0=OP.mult, op1=OP.add)
                    t2 = st.tile([128, 8 * DC], fp32, tag="st32", name="t2")
                    nc.vector.tensor_scalar_add(t2[:], t1[:], -MAGIC)
                    wb = pm.tile([128, 8 * DC], bf16, tag=f"wb{n}", name=f"wb{n}")
                    nc.vector.tensor_scalar(
                        wb[:], t2[:], 1.0, -1.0, op0=OP.min, op1=OP.max)
                    w_bf[n] = wb
                    if debug and n == "q":
                        nc.gpsimd.dma_start(d_wb[:], wb[:])
                for k in range(2):
                    wt = st.tile([128, D], fp32, tag="st32", name="wt")
                    nc.sync.dma_start(wt[:], wo[k * 128 : (k + 1) * 128, :])
                    t1 = st.tile([128, D], fp32, tag="st32", name="t1")
                    nc.vector.tensor_scalar(
                        t1[:], wt[:], scB[:, 4:5], MAGIC, op0=OP.mult, op1=OP.add)
                    t2 = st.tile([128, D], fp32, tag="st32", name="t2")
                    nc.vector.tensor_scalar_add(t2[:], t1[:], -MAGIC)
                    nc.vector.tensor_scalar(
                        wo_bf[k][:], t2[:], 1.0, -1.0, op0=OP.min, op1=OP.max)

                xq = []
                for i in range(8):
                    xt = st.tile([128, S], fp32, tag="st32", name="xt")
                    nc.sync.dma_start(xt[:], xT[i * 128 : (i + 1) * 128, :])
                    t1 = st.tile([128, S], fp32, tag="st32", name="t1")
                    nc.vector.tensor_scalar(
                        t1[:], xt[:], scB[:, 0:1], MAGIC, op0=OP.mult, op1=OP.add)
                    xb = pm.tile([128, S], bf16, tag=f"xq{i}", name=f"xq{i}")
                    nc.vector.tensor_scalar_add(xb[:], t1[:], -MAGIC)
                    xq.append(xb)
                    if debug and i == 0:
                        nc.gpsimd.dma_start(d_xq[:], xb[:])

                # scaled rope tables (bf16)
                for j, n in enumerate(("q", "k")):
                    for src_ext, dst in ((cosT, rope_bf[n][0]),
                                         (sinT, rope_bf[n][1])):
                        tf = st.tile([128, S], fp32, tag="st32", name="tf")
                        nc.sync.dma_start(tf[:], src_ext[:])
                        nc.vector.tensor_scalar_mul(
                            dst[:], tf[:], scB[:, 5 + j : 6 + j])

                # ---------- qkv projections ----------
                q_int = [pm.tile([128, S], bf16, tag=f"qi{t}", name=f"qi{t}")
                         for t in range(2)]
                k_int = [pm.tile([128, S], bf16, tag=f"ki{t}", name=f"ki{t}")
                         for t in range(2)]
                with tc.tile_pool(name="psq", bufs=4, space="PSUM") as psq:
                    for n, dest in (("q", q_int), ("k", k_int)):
                        for m in range(2):
                            for nn in range(NQ):
                                ps = psq.tile([128, QW], fp32, tag="ps", name="ps")
                                for kk in range(8):
                                    nc.tensor.matmul(
                                        ps[:],
                                        w_bf[n][:, kk * DC + m * 128
                                                : kk * DC + (m + 1) * 128],
                                        xq[kk][:, nn * QW : (nn + 1) * QW],
                                        start=(kk == 0), stop=(kk == 7),
                                    )
                                nc.vector.tensor_copy(
                                    dest[m][:, nn * QW : (nn + 1) * QW], ps[:])
                    for ci in range(KC):
                        ps = psq.tile([128, DC], fp32, tag="psv", name="psv")
                        for kk in range(8):
                            nc.tensor.matmul(
                                ps[:],
                                xq[kk][:, ci * 128 : (ci + 1) * 128],
                                w_bf["v"][:, kk * DC : (kk + 1) * DC],
                                start=(kk == 0), stop=(kk == 7),
                            )
                        vt = v_sm[ci]
                        for hh in range(HPC):
                            nc.vector.tensor_scalar_mul(
                                vt[:, V_DST[hh] : V_DST[hh] + 64],
                                ps[:, hh * 64 : (hh + 1) * 64], scB[:, 7:8])
                        if debug and ci == 0:
                            nc.gpsimd.dma_start(d_v[:], vt[:])
                    for ci in range(KC):
                        nc.vector.memset(v_sm[ci][:, 64:128], 1.0)
                        nc.vector.memset(v_sm[ci][:, 320:384], 1.0)

                # ---------- rope (bf16) ----------
                for n, src, dst in (("q", q_int, qr), ("k", k_int, kr)):
                    cb, sb = rope_bf[n]
                    for t in range(2):
                        sh = st.tile([128, S], bf16, tag="stbf", name="sh")
                        for blk in range(4):
                            sblk = (blk ^ 1) * 32
                            nc.scalar.dma_start(
                                sh[blk * 32 : blk * 32 + 32, :],
                                src[t][sblk : sblk + 32, :])
                        t1 = st.tile([128, S], bf16, tag="stbf", name="t1b")
                        nc.vector.tensor_mul(t1[:], src[t][:], cb[:])
                        t2 = st.tile([128, S], bf16, tag="stbf", name="t2b")
                        nc.vector.tensor_mul(t2[:], sh[:], sb[:])
                        nc.vector.tensor_add(dst[t][:], t1[:], t2[:])
                        if debug and n == "q" and t == 0:
                            nc.gpsimd.dma_start(d_qi[:], src[t][:])
                            nc.gpsimd.dma_start(d_qr[:], dst[t][:])

            # ---------- attention ----------
            with (
                tc.tile_pool(name="pss", bufs=2, space="PSUM") as pss,
                tc.tile_pool(name="psav", bufs=1, space="PSUM") as psav,
                tc.tile_pool(name="expp", bufs=KC + 2) as expp,
                tc.tile_pool(name="wnp", bufs=3) as wnp,
                tc.tile_pool(name="rsp", bufs=1) as rsp,
            ):
                for h in range(HPC):
                    tp, half = h // 2, (h % 2) * 64
                    ao_base, sum_base = (0, 64) if h % 2 == 0 else (64, 0)
                    pav = psav.tile([128, S], fp32, tag="pav", name="pav")
                    exp_tiles = []
                    for kc in range(KC):
                        et = expp.tile([128, S], bf16, tag="exp", name=f"exp{kc}")
                        exp_tiles.append(et)
                        for ne in range(NE):
                            ps = pss.tile([128, EXPW], fp32, tag="pss", name="ps")
                            for nqq in range(EXPW // QW):
                                n = ne * (EXPW // QW) + nqq
                                nc.tensor.matmul(
                                    ps[:, nqq * QW : (nqq + 1) * QW],
                                    kr[tp][half : half + 64,
                                           kc * 128 : (kc + 1) * 128],
                                    qr[tp][half : half + 64,
                                           n * QW : (n + 1) * QW],
                                    start=True, stop=True,
                                )
                            nc.scalar.activation(
                                et[:, ne * EXPW : (ne + 1) * EXPW], ps[:],
                                ACTF.Exp, scale=0.125)
                            if debug and h == 0 and kc == 0 and ne == 0:
                                nc.gpsimd.dma_start(d_exp[:, 0 : EXPW], et[:, 0 : EXPW])
                        for n in range(NQ):
                            nc.tensor.matmul(
                                pav[:, n * QW : (n + 1) * QW],
                                v_sm[kc][:, V_LHS[h] : V_LHS[h] + 128],
                                et[:, n * QW : (n + 1) * QW],
                                start=(kc == 0), stop=(kc == KC - 1),
                            )
                    rs64 = rsp.tile([64, S], fp32, tag="rs64", name="rs64")
                    nc.scalar.copy(rs64[:], pav[sum_base : sum_base + 64, :])
                    rc64 = rsp.tile([64, S], fp32, tag="rc64", name="rc64")
                    rscr = rsp.tile([64, S], fp32, tag="rscr", name="rscr")
                    nc.vector.reciprocal_approx_accurate(
                        rc64[:], rs64[:], rscr[:])
                    if debug and h == 0:
                        nc.sync.dma_start(d_rc[:], rc64[:])
                    nc.vector.tensor_mul(
                        att_oT[tp][half : half + 64, :],
                        pav[ao_base : ao_base + 64, :], rc64[:])
                    rcb = rsp.tile([128, S], bf16, tag="rcb", name="rcb")
                    if debug and h == 1:
                        nc.sync.dma_start(d_ao[:], att_oT[0][:])
                    nc.vector.tensor_copy(rcb[0:64, :], rc64[:])
                    nc.vector.tensor_copy(rcb[64:128, :], rc64[:])
                    for kc in range(KC):
                        wn = wnp.tile([128, S], bf16, tag="wn", name="wn")
                        nc.vector.tensor_mul(wn[:], exp_tiles[kc][:], rcb[:])
                        nc.gpsimd.dma_start(
                            attw[h, kc * 128 : (kc + 1) * 128, :], wn[:])

            # ---------- attn_out quantization ----------
            am_cols = mp.tile([128, 2], fp32)
            for t in range(2):
                nc.vector.tensor_reduce(
                    am_cols[:, t : t + 1], att_oT[t][:], axis=AX.X, op=OP.max,
                    apply_absolute_value=True)
            am1 = mp.tile([128, 1], fp32)
            nc.vector.tensor_reduce(am1[:], am_cols[:], axis=AX.X, op=OP.max)
            am_r = mp.tile([128, 1], fp32)
            nc.gpsimd.partition_all_reduce(
                am_r[:], am1[:], channels=128, reduce_op=bass_isa.ReduceOp.max)
            am_in_sb = mp.tile([1, 1], fp32)
            nc.vector.tensor_copy(am_in_sb[:], am_r[0:1, :])
            nc.sync.dma_start(b_am_in[:], am_in_sb[:])
            nc.gpsimd.collective_compute(
                "AllReduce", OP.max, replica_groups=RG,
                ins=[b_am_in[:]], outs=[b_am_out[:]])
            am_sb = mp.tile([1, 1], fp32)
            nc.sync.dma_start(am_sb[:], b_am_out[:])

            sao_sb = mp.tile([1, 1], fp32)
            nc.vector.tensor_scalar_mul(sao_sb[:], am_sb[:], 1.0 / 127.0)
            rao_sb = mp.tile([1, 1], fp32)
            nc.vector.reciprocal(rao_sb[:], sao_sb[:])
            scv2 = mp.tile([1, 2], fp32)
            nc.vector.tensor_copy(scv2[:, 0:1], rao_sb[:])
            nc.vector.tensor_mul(scv2[:, 1:2], sao_sb[:], sw_sb[:, 3:4])
            scB2 = mp.tile([128, 2], fp32)
            nc.gpsimd.partition_broadcast(scB2[:], scv2[:], channels=128)

            with tc.tile_pool(name="tail", bufs=2) as tl:
                for t in range(2):
                    t1 = tl.tile([128, S], fp32, tag="aotmp", name="aot")
                    nc.vector.tensor_scalar(
                        t1[:], att_oT[t][:], scB2[:, 0:1], MAGIC,
                        op0=OP.mult, op1=OP.add)
                    nc.vector.tensor_scalar_add(aoq[t][:], t1[:], -MAGIC)

                # ---------- output projection (partial sums) ----------
                with tc.tile_pool(name="psw", bufs=4, space="PSUM") as psw:
                    for m in range(8):
                        ys = tl.tile([128, S], fp32, tag="ys", name="ys")
                        for n in range(NQ):
                            ps = psw.tile([128, QW], fp32, tag="psw", name="ps")
                            for kk in range(2):
                                nc.tensor.matmul(
                                    ps[:],
                                    wo_bf[kk][:, m * 128 : (m + 1) * 128],
                                    aoq[kk][:, n * QW : (n + 1) * QW],
                                    start=(kk == 0), stop=(kk == 1),
                                )
                            nc.vector.tensor_scalar_mul(
                                ys[:, n * QW : (n + 1) * QW], ps[:], scB2[:, 1:2])
                        nc.sync.dma_start(yT[m * 128 : (m + 1) * 128, :], ys[:])

    nc.compile()
    return nc


def _prepare_inputs(hidden_states, Wq, Wk, Wv, Wo, S):
    inv_freq = 1.0 / (ROPE_BASE ** (np.arange(0, DKV, 2, dtype=np.float32) / DKV))
    t = np.arange(S, dtype=np.float32)
    freqs = np.einsum("i,j->ij", t, inv_freq).astype(np.float32)
    emb = np.concatenate([freqs, freqs], axis=-1)
    cosT1 = np.cos(emb).T.astype(np.float32)
    sinT1 = np.sin(emb).T.astype(np.float32)
    sgn = np.where(np.arange(DKV) < DKV // 2, -1.0, 1.0).astype(np.float32)
    sinT1 = sinT1 * sgn[:, None]
    cos2 = np.ascontiguousarray(np.concatenate([cosT1, cosT1], axis=0))
    sin2 = np.ascontiguousarray(np.concatenate([sinT1, sinT1], axis=0))

    def pack_w(wT):  # [1024, 256] -> [128, 8*256] chunk-interleaved
        return np.ascontiguousarray(
            wT.reshape(8, 128, DC).transpose(1, 0, 2).reshape(128, 8 * DC))

    in_maps = []
    for c in range(N_CORES):
        b, g = c // 4, c % 4
        cols = slice(g * DC, (g + 1) * DC)
        in_maps.append({
            "xT": np.ascontiguousarray(hidden_states[b].T),
            "wq": pack_w(np.ascontiguousarray(Wq[cols, :].T)),
            "wk": pack_w(np.ascontiguousarray(Wk[cols, :].T)),
            "wv": pack_w(np.ascontiguousarray(Wv[cols, :].T)),
            "wo": np.ascontiguousarray(Wo[:, cols].T),
            "cosT": cos2,
            "sinT": sin2,
        })
    return in_maps


def run_sharded(hidden_states, Wq, Wk, Wv, Wo, trace=False):
    from concourse.bass_utils import run_bass_kernel_spmd

    Sv = hidden_states.shape[1]
    if "nc" not in _CACHE or _CACHE.get("S") != Sv:
        _CACHE["nc"] = build_program(Sv)
        _CACHE["S"] = Sv
    nc = _CACHE["nc"]
    in_maps = _prepare_inputs(hidden_states, Wq, Wk, Wv, Wo, Sv)
    res = run_bass_kernel_spmd(nc, in_maps, list(range(N_CORES)), trace=trace)

    attn_w = np.empty((B, H, Sv, Sv), dtype=np.float32)
    out = np.zeros((B, Sv, D), dtype=np.float32)
    for c in range(N_CORES):
        b, g = c // 4, c % 4
        aw = np.asarray(res.results[c]["attw"]).astype(np.float32)  # [4, Sk, Sq]
        for h in range(HPC):
            attn_w[b, g * HPC + h] = aw[h].T
        out[b] += np.asarray(res.results[c]["yT"]).T
    return (out, attn_w), res


def kernel(hidden_states, Wq, Wk, Wv, Wo):
    hidden_states = np.asarray(hidden_states, dtype=np.float32)
    (out, attn_w), _ = run_sharded(
        hidden_states,
        np.asarray(Wq, dtype=np.float32), np.asarray(Wk, dtype=np.float32),
        np.asarray(Wv, dtype=np.float32), np.asarray(Wo, dtype=np.float32),
    )
    return out, attn_w
